# revision 35
# baseline (speedup 1.0000x reference)
"""Trainium2 Bass kernel for BiAttention (b=8, n=m=1024, d=512).

Sharding: data-parallel over batch — one batch element per NeuronCore,
8 cores, no cross-core communication.

Per-core algorithm (softmax shift-invariance lets the Linear(3d,1) row/col
terms, the bias, and both padding masks fold into per-row/col exponent
weights, so no max-subtraction pass and no partition-axis reductions are
needed; logits are ~N(0,1) so raw exp is safe):

  sim      = (x1*w3) @ x2^T                     (n, m)   [tri term only]
  s1[n]    = x1 @ w1,   s2[m] = x2 @ w2
  g1[n]    = exp(s1 + (-30000 if x1_mask else 0))
  g2[m]    = exp(s2 + (-30000 if x2_mask else 0))
  E_raw    = exp(sim);  E_g = E_raw * g1[:,None];  ET_g = E_raw^T * g2[:,None]
  U_row    = ET_g^T @ x2          (n, d);  den1[n] = sum_m ET_g[m,n]
  U_col    = E_g^T  @ x1          (m, d);  den2[m] = sum_n E_g[n,m]
  c2q      = U_row / den1
  q2c      = U_col / den2
  V        = ET_g^T @ q2c         (n, d)
  q2c_att  = V / den1
  out      = [x1, c2q, x1*c2q, x1*q2c_att]      (n, 4d)

All big matmuls run in float32r (full-rate PE). s1 is recovered from the
w3-scaled transposed copy via u1 = w1/w3.

Mask-suffix specialization: rows/cols whose g weight is 0 contribute
nothing to any weighted sum, so tiles of 128 that are FULLY masked at the
end of either sequence can be skipped in the contractions. The host
inspects the masks at call time and dispatches to a NEFF compiled for
(kn, km) = (# n-tiles with any valid row, # m-tiles with any valid col).
Partially-masked tiles are still handled exactly via the exponent biases,
so any mask pattern is computed correctly (a previously-unseen (kn, km)
pair just triggers a one-time compile).
"""

import numpy as np
from contextlib import ExitStack

import concourse.bacc as bacc
import concourse.tile as tile
import concourse.mybir as mybir
from concourse.bass_utils import run_bass_kernel_spmd
from concourse.masks import make_identity

F32 = mybir.dt.float32
U8 = mybir.dt.uint8
R = mybir.dt.float32r
EXP = mybir.ActivationFunctionType.Exp
COPY = mybir.ActivationFunctionType.Copy

P = 128
N = 1024          # x1 rows
M = 1024          # x2 rows
D = 512           # feature dim
NT, MT, DC = N // P, M // P, D // P
NEGB = -30000.0   # exp(x + NEGB) == 0.0 exactly for |x| < 80

N_CORES = 8

_CACHE = {}


def _chunks(width):
    """Split [0, width) into 512-wide pieces (remainder last)."""
    out = []
    o = 0
    while o < width:
        w = min(512, width - o)
        out.append((o, w))
        o += w
    return out


def _build(kn, km, mm_dtype=R):
    """Build the kernel keeping the first kn n-tiles / km m-tiles of the
    contractions (tiles beyond that must be fully masked)."""
    vm = km * P  # valid m extent
    nc = bacc.Bacc("TRN2", target_bir_lowering=False, debug=False)
    x1d = nc.dram_tensor("x1", [N, D], F32, kind="ExternalInput").ap()
    x2d = nc.dram_tensor("x2", [M, D], F32, kind="ExternalInput").ap()
    m1d = nc.dram_tensor("x1_mask", [N], U8, kind="ExternalInput").ap()
    m2d = nc.dram_tensor("x2_mask", [M], U8, kind="ExternalInput").ap()
    wd = nc.dram_tensor("W", [3 * D], F32, kind="ExternalInput").ap()
    outd = nc.dram_tensor("out", [N, 4 * D], F32, kind="ExternalOutput").ap()

    x1r_d = x1d.rearrange("(t p) d -> p t d", p=P)
    x2r_d = x2d.rearrange("(t p) d -> p t d", p=P)
    out_r = outd.rearrange("(t p) e -> p t e", p=P)

    with tile.TileContext(nc) as tc, ExitStack() as ctx:
        const = ctx.enter_context(tc.tile_pool(name="const", bufs=1))
        big = ctx.enter_context(tc.tile_pool(name="big", bufs=1))
        rows = ctx.enter_context(tc.tile_pool(name="rows", bufs=1))
        # large variants (km near MT) need the SBUF back from the staging pools
        work = ctx.enter_context(tc.tile_pool(name="work", bufs=3 if km <= 6 else 2))
        x2p = ctx.enter_context(tc.tile_pool(name="x2p", bufs=2 if km <= 6 else 1))
        psb = ctx.enter_context(tc.tile_pool(name="psb", bufs=2, space="PSUM"))
        pss = ctx.enter_context(tc.tile_pool(name="pss", bufs=4, space="PSUM"))
        psd = ctx.enter_context(tc.tile_pool(name="psd", bufs=2, space="PSUM"))

        # ---------- constants ----------
        ident = const.tile([P, P], F32)
        make_identity(nc, ident)
        # W: one contiguous row load, then PE row->column transposes (a strided
        # (c p)->p c DMA would issue 1536 4-byte packets and hog the DMA engines)
        wrow = rows.tile([1, 12 * P], F32)
        nc.sync.dma_start(wrow[:], wd.rearrange("(a n) -> a n", a=1))
        pwc = psd.tile([P, 12], F32, tag="ps_d")
        for c in range(12):
            nc.tensor.transpose(pwc[:, c:c + 1], wrow[0:1, c * P:(c + 1) * P],
                                ident[0:1, 0:1])
        wcols = const.tile([P, 12], F32)  # (p, c): w1=0:4 w2=4:8 w3=8:12
        nc.vector.tensor_copy(wcols[:], pwc[:])
        w3rec = const.tile([P, 4], F32)
        nc.vector.reciprocal(w3rec[:], wcols[:, 8:12])
        u1r = const.tile([P, 4], mm_dtype)  # w1/w3 — recovers s1 from x1w3T
        nc.vector.tensor_mul(u1r[:], wcols[:, 0:4], w3rec[:])
        w2r = const.tile([P, 4], mm_dtype)
        nc.vector.tensor_copy(w2r[:], wcols[:, 4:8])
        ones_f = const.tile([P, 1], F32)
        nc.vector.memset(ones_f[:], 1.0)
        ones_r = const.tile([P, 1], mm_dtype)
        nc.vector.tensor_copy(ones_r[:], ones_f[:])
        identr = const.tile([P, P], mm_dtype)
        nc.vector.tensor_copy(identr[:], ident[:])

        # masks -> (1, N) exponent offsets (0 valid / NEGB padded)
        m1row = rows.tile([1, N], U8)
        nc.sync.dma_start(m1row[:], m1d.rearrange("(a n) -> a n", a=1))
        m2row = rows.tile([1, M], U8)
        nc.sync.dma_start(m2row[:], m2d.rearrange("(a n) -> a n", a=1))
        logm1 = rows.tile([1, N], F32)
        nc.vector.tensor_scalar_mul(logm1[:], m1row[:], NEGB)
        logm2 = rows.tile([1, M], F32)
        nc.vector.tensor_scalar_mul(logm2[:], m2row[:], NEGB)

        x1n = big.tile([P, NT, D], F32)          # natural x1 (outputs)
        x1w3T = big.tile([P, DC, N], mm_dtype)   # (d_chunk, n) of x1*w3
        x2T = big.tile([P, DC, vm], mm_dtype)    # (d_chunk, m<vm) of x2
        x1aug = big.tile([P, NT, D], mm_dtype)   # f32r x1; later x1*rden1
        x2aug = big.tile([P, km, D], mm_dtype)   # f32r x2 (kept tiles)

        nc.sync.dma_start(x1n[:, 0:4, :], x1r_d[:, 0:4, :])

        def x1_quad(q):
            for c in range(DC):
                pq = pss.tile([P, 512], F32, tag="ps_sm", name=f"x1q_{q}_{c}")
                for j in range(4):
                    nc.tensor.transpose(pq[:, j * P:(j + 1) * P],
                                        x1n[:, q * 4 + j, c * P:(c + 1) * P],
                                        ident[:])
                # evict fused with w3 scaling (per-partition in (d, n) layout)
                nc.vector.tensor_scalar_mul(
                    x1w3T[:, c, q * 512:(q + 1) * 512], pq[:], wcols[:, 8 + c:9 + c])
            for j in range(4):
                nc.scalar.copy(x1aug[:, q * 4 + j, :], x1n[:, q * 4 + j, :])

        def x2_dma(q):
            jw = min(4, km - q * 4)
            x2q = x2p.tile([P, 4, D], F32, tag="x2s", name=f"x2t_{q}")
            nc.sync.dma_start(x2q[:, 0:jw, :], x2r_d[:, q * 4:q * 4 + jw, :])
            return x2q

        def x2_quad(q, x2q):
            jw = min(4, km - q * 4)
            for c in range(DC):
                pq = pss.tile([P, jw * P], F32, tag="ps_sm", name=f"x2q_{q}_{c}")
                for j in range(jw):
                    nc.tensor.transpose(pq[:, j * P:(j + 1) * P],
                                        x2q[:, j, c * P:(c + 1) * P], ident[:])
                nc.scalar.copy(x2T[:, c, q * 512:q * 512 + jw * P], pq[:])
            for j in range(jw):
                nc.scalar.copy(x2aug[:, q * 4 + j, :], x2q[:, j, :])

        x2q0 = x2_dma(0)
        x1_quad(0)
        x2_quad(0, x2q0)
        nc.sync.dma_start(x1n[:, 4:8, :], x1r_d[:, 4:8, :])

        # softmax weight columns (g2c gates only the ET evictions)
        b1col = const.tile([P, NT], F32)
        b2col = const.tile([P, km], F32)
        g1c = const.tile([P, NT], F32)
        g2c = const.tile([P, km], F32)

        E = big.tile([P, NT, vm], mm_dtype)   # exp(sim); scaled to E_g in place
        ET = big.tile([P, km, N], mm_dtype)   # exp(sim)^T * g2

        mch = _chunks(vm)

        def sim_tile(t, h):
            off, w = mch[h]
            pe = psb.tile([P, w], F32, tag="ps_big", name=f"pe_{t}_{h}")
            for c in range(DC):
                nc.tensor.matmul(pe[:],
                                 x1w3T[:, c, t * P:(t + 1) * P],
                                 x2T[:, c, off:off + w],
                                 start=(c == 0), stop=(c == DC - 1))
            nc.scalar.activation(E[:, t, off:off + w], pe[:], EXP)

        def e_quad(u, tq):
            # transpose 4 E n-tiles at m-slice u (f32r, 1.5 cyc/row), evict * g2
            pq = pss.tile([P, 512], mm_dtype, tag="ps_sm", name=f"eq_{u}_{tq}")
            for j in range(4):
                nc.tensor.transpose(pq[:, j * P:(j + 1) * P],
                                    E[:, tq * 4 + j, u * P:(u + 1) * P],
                                    identr[:])
            nc.scalar.activation(ET[:, u, tq * 512:(tq + 1) * 512], pq[:],
                                 COPY, scale=g2c[:, u:u + 1])

        def s_and_g():
            # s1/s2 rows -> bias columns (PE row->col transpose) -> g = exp
            for (name, lhs, rhsT, wid, logm, bcol, gcol, nt) in (
                ("b1", u1r, x1w3T, N, logm1, b1col, g1c, NT),
                ("b2", w2r, x2T, vm, logm2, b2col, g2c, km),
            ):
                brow = rows.tile([1, wid], F32, tag="rowbuf", name=f"{name}row")
                for h, (off, w) in enumerate(_chunks(wid)):
                    ps_s = psd.tile([1, w], F32, tag="ps_d", name=f"ps_{name}_{h}")
                    for c in range(DC):
                        nc.tensor.matmul(ps_s[:], lhs[:, c:c + 1],
                                         rhsT[:, c, off:off + w],
                                         start=(c == 0), stop=(c == DC - 1))
                    nc.vector.tensor_add(brow[:, off:off + w], ps_s[:],
                                         logm[:, off:off + w])
                pbc = psd.tile([P, nt], F32, tag="ps_d", name=f"pbc_{name}")
                for t in range(nt):
                    nc.tensor.transpose(pbc[:, t:t + 1],
                                        brow[0:1, t * P:(t + 1) * P],
                                        ident[0:1, 0:1])
                nc.vector.tensor_copy(bcol[:], pbc[:])
                nc.scalar.activation(gcol[:], bcol[:], EXP)

        # Interleave sim (real matmuls keep the HAM clock warm) with the
        # transpose batches, in dependency-feasible order.
        nh = len(mch)
        for t in (0, 1, 2, 3):
            sim_tile(t, 0)
        if km > 4:
            x2_quad(1, x2_dma(1))
        for h in range(1, nh):
            for t in (0, 1, 2, 3):
                sim_tile(t, h)
        x1_quad(1)
        # out block 0 = x1: one bulk store, now that x1n is fully loaded
        nc.sync.dma_start(out_r[:, :, 0:D], x1n[:])
        for h in range(nh):
            sim_tile(4, h)
        s_and_g()
        for h in range(nh):
            sim_tile(5, h)
        for u in range(km // 2):
            e_quad(u, 0)
        for h in range(nh):
            sim_tile(6, h)
        for u in range(km // 2, km):
            e_quad(u, 0)
        for h in range(nh):
            sim_tile(7, h)
        # E_g = E_raw * g1 in place for fully-transposed, kept tiles
        for t in range(min(4, kn)):
            nc.vector.tensor_scalar_mul(E[:, t, :], E[:, t, :], g1c[:, t:t + 1])
        for u in range(km):
            e_quad(u, 1)
        for t in range(4, kn):
            nc.vector.tensor_scalar_mul(E[:, t, :], E[:, t, :], g1c[:, t:t + 1])

        # ---------- denominators (ones-vector row matmuls + PE transpose) ----------
        rden1 = const.tile([P, NT], F32)
        rden2 = const.tile([P, km], F32)

        def den_mms(name, Esrc, kk, wid):
            drow = rows.tile([1, wid], F32, tag="rowbuf", name=f"{name}row")
            for h, (off, w) in enumerate(_chunks(wid)):
                ps_d = psd.tile([1, w], F32, tag="ps_d", name=f"ps_{name}_{h}")
                for k in range(kk):
                    nc.tensor.matmul(ps_d[:], ones_r[:], Esrc[:, k, off:off + w],
                                     start=(k == 0), stop=(k == kk - 1))
                nc.vector.tensor_copy(drow[:, off:off + w], ps_d[:])
            return drow

        def den_fin(name, drow, rden, nt):
            pdc = psd.tile([P, nt], F32, tag="ps_d", name=f"pdc_{name}")
            for t in range(nt):
                nc.tensor.transpose(pdc[:, t:t + 1], drow[0:1, t * P:(t + 1) * P],
                                    ident[0:1, 0:1])
            nc.vector.reciprocal(rden[:], pdc[:])

        # ---------- U_col -> q2c (plain, f32r) ----------
        drow2 = den_mms("den2", E, kn, vm)
        den_fin("den2", drow2, rden2, km)
        Q2C = big.tile([P, km, D], mm_dtype)
        for u in range(km):
            pu = pss.tile([P, D], F32, tag="ps_sm", name=f"pu_{u}")
            for k in range(kn):
                nc.tensor.matmul(pu[:], E[:, k, u * P:(u + 1) * P], x1aug[:, k, :],
                                 start=(k == 0), stop=(k == kn - 1))
            nc.scalar.activation(Q2C[:, u, :], pu[:], COPY, scale=rden2[:, u:u + 1])
            if u == 2:
                drow1 = den_mms("den1", ET, km, N)

        # ---------- U_row -> c2q ; out blocks 0, 1, 2 ----------
        den_fin("den1", drow1, rden1, NT)
        for t in range(NT):
            pr = pss.tile([P, D], F32, tag="ps_sm", name=f"pr_{t}")
            for k in range(km):
                nc.tensor.matmul(pr[:], ET[:, k, t * P:(t + 1) * P], x2aug[:, k, :],
                                 start=(k == 0), stop=(k == km - 1))
            # blocks 1+2 staged contiguously -> one store per tile (SP-queue
            # issue time was pacing the PE here at 3 DMAs/tile)
            combo = work.tile([P, 2 * D], F32, tag="ev", name=f"cb_{t}")
            nc.scalar.activation(combo[:, 0:D], pr[:], COPY,
                                 scale=rden1[:, t:t + 1])
            nc.vector.tensor_mul(combo[:, D:2 * D], x1n[:, t, :], combo[:, 0:D])
            nc.sync.dma_start(out_r[:, t, D:3 * D], combo[:])

        # ---------- V -> q2c_att ; out block 3 = (x1*rden1) . V ----------
        # x1aug is dead after U_col — reuse it as x1 * rden1 so block 3 is a
        # single tensor_tensor per tile
        for t in range(NT):
            pv = pss.tile([P, D], F32, tag="ps_sm", name=f"pv_{t}")
            for k in range(km):
                nc.tensor.matmul(pv[:], ET[:, k, t * P:(t + 1) * P], Q2C[:, k, :],
                                 start=(k == 0), stop=(k == km - 1))
            nc.vector.tensor_scalar_mul(x1aug[:, t, :], x1n[:, t, :],
                                        rden1[:, t:t + 1])
            prod = work.tile([P, D], F32, tag="ev", name=f"pv2_{t}")
            nc.vector.tensor_mul(prod[:], x1aug[:, t, :], pv[:])
            nc.sync.dma_start(out_r[:, t, 3 * D:4 * D], prod[:])

    nc.compile()
    return nc


def _kept_tiles(mask):
    """Tiles (of 128) up to and including the last one with any valid row."""
    valid = ~mask.astype(bool)           # (b, L)
    any_valid = valid.reshape(valid.shape[0], -1, P).any(axis=2).any(axis=0)
    nz = np.nonzero(any_valid)[0]
    return int(nz[-1]) + 1 if len(nz) else 1


def _get_nc(kn, km):
    key = (kn, km)
    if key not in _CACHE:
        _CACHE[key] = _build(kn, km)
    return _CACHE[key]


def _run(inputs, trace=False, trace_cores=None):
    x1 = np.ascontiguousarray(np.asarray(inputs["x1"], dtype=np.float32))
    x2 = np.ascontiguousarray(np.asarray(inputs["x2"], dtype=np.float32))
    m1 = np.ascontiguousarray(np.asarray(inputs["x1_mask"]).astype(np.uint8))
    m2 = np.ascontiguousarray(np.asarray(inputs["x2_mask"]).astype(np.uint8))
    W = np.ascontiguousarray(np.asarray(inputs["W"], dtype=np.float32))
    nc = _get_nc(_kept_tiles(m1), _kept_tiles(m2))
    in_maps = [
        {"x1": x1[i], "x2": x2[i], "x1_mask": m1[i], "x2_mask": m2[i], "W": W}
        for i in range(N_CORES)
    ]
    res = run_bass_kernel_spmd(nc, in_maps, core_ids=list(range(N_CORES)),
                               trace=trace, trace_cores=trace_cores)
    out = np.stack([res.results[i]["out"] for i in range(N_CORES)], axis=0)
    return out.astype(np.float32), res


def kernel(x1, x1_mask, x2, x2_mask, W, bias=None, **_kw):
    # bias is mathematically irrelevant: a global additive constant cancels in
    # both softmaxes, and every output term is softmax-weighted.
    out, _ = _run({"x1": x1, "x1_mask": x1_mask, "x2": x2, "x2_mask": x2_mask,
                   "W": W})
    return out
